# revision 1
# baseline (speedup 1.0000x reference)
"""Single-head causal attention (B=256, T=256, C=1024, D=64) on 8 TRN2 NeuronCores.

Data-parallel over batch (32 batches/core). The per-core x traffic is the
roofline (~16.8MB at ~360B/ns of serialized DMA-engine time = ~47us); the
schedule keeps the DMA stream gapless and holds PE/DVE/ACT/Pool well under
the DMA cadence:

  * x ships as an fp8-e4m3 hi/lo pair (x = xh + xl, xl the unscaled
    residual), pre-transposed to partition-major [C, T]. Same bytes as bf16,
    but the projections run in DoubleRow fp8 mode (4x bf16 PE throughput),
    contracting two 128-chunks per instruction:
        q|k^T [128,T]: xh@wh + xl@wh            (2 terms, 8 matmuls/batch)
        v     [T,D]:   xh@wh + xl@wh + xh@wl    (3 terms, 24 matmuls/batch)
    (v keeps the third term -- its error feeds the output directly; q/k only
    perturb softmax weights, rel err ~9e-3 total vs the 2e-2 gate.)
    Weights are host-prescaled by 8 so their fp8 hi/lo split stays in the
    normal range; the 8x comes out in the exp scale and the fused
    denominator column (memset 8.0).
    With 2 q/k terms the wqk-lo half is never referenced, so only wqk-hi
    ships and loads.
  * Superbatch pipeline (2 batches per stage), per iteration sb:
    qk(sb) -> [q copy on DVE | k copy on ACT] -> v(sb) ->
    scores^T(sb-1) in bf16 -> one ACT exp per batch -> Pool affine_select
    causal masks (two diagonal quadrants only) -> finale(sb-2):
    o' [T,2,D+1] = e^T.T @ [v | 8] + DVE reciprocal/multiply into a bf16
    staging tile (host upcasts to f32). Splitting the PSUM->SBUF q/k copies
    across DVE and ACT keeps the copy chain off the critical loop.
  * DMA: one 1MB x load per superbatch (8KB/partition contiguous), one 64KB
    store per superbatch, hi-only wqk + hi/lo wv weight loads; ~37 DMA
    instructions, all with >=512B descriptors (no sub-512B latency penalty).
    The first superbatch loads per batch (first batch additionally by hi/lo
    half) and the last per batch, shortening pipeline fill and drain.
"""

import numpy as np
import ml_dtypes

import concourse.bacc as bacc
import concourse.mybir as mybir
import concourse.tile as tile
from concourse.bass_utils import run_bass_kernel_spmd

B, T, C, D = 256, 256, 1024, 64
NCORES = 8
BPC = B // NCORES  # batches per core
NSB = BPC // 2  # superbatches (2 batches per DMA)
CCH = C // 128  # contraction chunks
NCP = CCH // 2  # chunk pairs (DoubleRow contracts 2 chunks/instruction)
WS = 8.0  # host weight prescale; keeps fp8 weight splits in normal range
SCALE = float(C) ** -0.5

BF16 = mybir.dt.bfloat16
F32 = mybir.dt.float32
F8 = mybir.dt.float8e4
E4M3 = ml_dtypes.float8_e4m3
DR = mybir.MatmulPerfMode.DoubleRow

TRACE = False
LAST_RESULT = None

# (x half, w half) term order: hi@hi, lo@hi, hi@lo
TERMS = ((0, 0), (1, 0), (0, 1))


def _build(pf=2, qk_terms=2, v_terms=3, mask_eng='affine', vcopy_eng='act'):
    nc = bacc.Bacc(
        "TRN2", target_bir_lowering=False, debug=False, num_devices=NCORES
    )
    # [sb, partition, batch-in-sb, hi/lo, chunk, t]
    xhl = nc.dram_tensor("xhl", [NSB, 128, 2, 2, CCH, T], F8, kind="ExternalInput")
    # wqk hi-only [partition, chunk, 128]; wv [partition, chunk, hi/lo, 64]
    wqk_d = nc.dram_tensor("wqk_d", [128, CCH, 128], F8, kind="ExternalInput")
    wv_d = nc.dram_tensor("wv_d", [128, CCH, 2, 64], F8, kind="ExternalInput")
    # [sb, partition, batch-in-sb, t-tile, d]
    out = nc.dram_tensor("out", [NSB, 128, 2, 2, D], BF16, kind="ExternalOutput")

    with tile.TileContext(nc) as tc:
        with (
            tc.tile_pool(name="singles", bufs=1) as singles,
            tc.tile_pool(name="xp", bufs=pf + 1) as xp,
            tc.tile_pool(name="sbp", bufs=3) as sbp,
            tc.tile_pool(name="ep", bufs=8) as ep,
            tc.tile_pool(name="vp", bufs=6) as vp,
            tc.tile_pool(name="stp", bufs=5) as stp,
            tc.tile_pool(name="rp", bufs=2) as rp,
            tc.tile_pool(name="qk_ps", bufs=2, space="PSUM") as qk_psp,
            tc.tile_pool(name="sc_ps", bufs=3, space="PSUM") as sc_psp,
            tc.tile_pool(name="v_ps", bufs=2, space="PSUM") as v_psp,
            tc.tile_pool(name="o_ps", bufs=1, space="PSUM") as o_psp,
        ):
            # qk weights load first: they gate the very first projection
            wqk_sb = singles.tile([128, CCH, 128], F8)
            nc.sync.dma_start(wqk_sb, wqk_d[:])
            wv_sb = singles.tile([128, CCH, 2, D], F8)

            # causal triangle (1 where s <= t within a 128-tile) built once;
            # masking is then a tensor-tensor multiply on any engine
            tri = singles.tile([128, 128], BF16)
            nc.gpsimd.memset(tri, 1.0)
            nc.gpsimd.affine_select(
                out=tri, in_=tri,
                compare_op=mybir.AluOpType.is_ge,
                fill=0.0, base=0, pattern=[[1, 128]], channel_multiplier=-1,
            )

            xt_tiles = {}

            def load_sb(k, split=False):
                t = xp.tile([128, 2, 2, CCH, T], F8, tag="xt")
                if split:
                    # per-batch halves: first batch's data (and compute)
                    # lands ~1.5us earlier at the pipeline head/tail
                    nc.sync.dma_start(t[:, 0], xhl[k][:, 0])
                    nc.sync.dma_start(t[:, 1], xhl[k][:, 1])
                else:
                    nc.sync.dma_start(t, xhl[k])
                xt_tiles[k] = t

            stages = {}

            def final_stage(sb, expT0, expT1, v_sb):
                """o' matmuls + softmax normalization for superbatch sb
                (emitted two superbatches late)."""
                o2 = o_psp.tile([128, 2, 2, D + 1], F32, tag="o_ps")
                for bi, expT in ((0, expT0), (1, expT1)):
                    nc.tensor.matmul(
                        o2[:, bi, 0], lhsT=expT[:, 0:128], rhs=v_sb[:, bi, 0],
                        start=True, stop=True,
                    )
                    nc.tensor.matmul(
                        o2[:, bi, 1], lhsT=expT[:, 128:256], rhs=v_sb[:, bi, 0],
                        start=True, stop=False,
                    )
                    nc.tensor.matmul(
                        o2[:, bi, 1], lhsT=expT[:, 256:384], rhs=v_sb[:, bi, 1],
                        start=False, stop=True,
                    )
                stages[sb] = stp.tile(
                    [128, 2, 2, D], BF16, tag="stage", name="stage"
                )
                # recip to SBUF first: engines may read only ONE PSUM
                # operand per instruction, and Pool can't read PSUM at all
                recip = rp.tile([128, 2, 2], F32, tag="recip")
                nc.vector.reciprocal(recip, o2[:, :, :, D])
                nc.vector.tensor_tensor(
                    stages[sb],
                    o2[:, :, :, 0:D],
                    recip[:, :, :, None].to_broadcast((128, 2, 2, D)),
                    mybir.AluOpType.mult,
                )

            def scores_stage(sb, q_sb, k_sb, v_sb, mask_mix=False):
                """scores^T + exp + causal mask for both batches of sb
                (emitted one superbatch late)."""
                expTs = []
                for bi in range(2):
                    # scores^T packed [128, 384]: cols 0:256 = (s<128, all t),
                    # 256:384 = (s>=128, t>=128); (s>=128, t<128) fully masked
                    sc_ps = sc_psp.tile([128, 3 * 128], F32, tag="sc")
                    nc.tensor.matmul(
                        sc_ps[:, 0:T],
                        lhsT=k_sb[:, bi, 0:128],
                        rhs=q_sb[:, bi],
                        start=True, stop=True,
                    )
                    nc.tensor.matmul(
                        sc_ps[:, T : T + 128],
                        lhsT=k_sb[:, bi, 128:T],
                        rhs=q_sb[:, bi, 128:T],
                        start=True, stop=True,
                    )
                    expT = ep.tile([128, 3 * 128], BF16, tag="expT")
                    nc.scalar.activation(
                        expT, sc_ps,
                        func=mybir.ActivationFunctionType.Exp,
                        scale=SCALE / (WS * WS),
                    )
                    for qi, quad in enumerate((0, 256)):
                        if mask_mix and qi == 0:
                            # drain only: DVE is idle there; halve the last
                            # serial mask chain by splitting across engines
                            nc.vector.tensor_tensor(
                                expT[:, quad : quad + 128],
                                expT[:, quad : quad + 128],
                                tri, mybir.AluOpType.mult,
                            )
                            continue
                        if mask_eng == 'affine':
                            nc.gpsimd.affine_select(
                                out=expT[:, quad : quad + 128],
                                in_=expT[:, quad : quad + 128],
                                compare_op=mybir.AluOpType.is_ge,
                                fill=0.0, base=0, pattern=[[1, 128]],
                                channel_multiplier=-1,
                            )
                            continue
                        if mask_eng == 'dve':
                            eng = nc.vector
                        elif mask_eng == 'pool':
                            eng = nc.gpsimd
                        else:  # mix: one quadrant each
                            eng = nc.vector if qi == 0 else nc.gpsimd
                        eng.tensor_tensor(
                            expT[:, quad : quad + 128],
                            expT[:, quad : quad + 128],
                            tri,
                            mybir.AluOpType.mult,
                        )
                    expTs.append(expT)
                return (sb, expTs[0], expTs[1], v_sb)

            # first batch split hi/lo: its hi-term matmuls start earlier
            t0 = xp.tile([128, 2, 2, CCH, T], F8, tag="xt", name="t0")
            nc.sync.dma_start(t0[:, 0, 0], xhl[0][:, 0, 0])
            nc.sync.dma_start(t0[:, 0, 1], xhl[0][:, 0, 1])
            nc.sync.dma_start(t0[:, 1], xhl[0][:, 1])
            xt_tiles[0] = t0
            nc.sync.dma_start(wv_sb, wv_d[:])
            load_sb(1, split=True)
            for k in range(2, min(pf, NSB)):
                load_sb(k)

            pend_sc = None  # superbatch sb-1: awaiting scores/exp/mask
            fin_q = []  # superbatches sb-2, sb-3: awaiting o'/normalize
            for sb in range(NSB):
                if sb + pf < NSB:
                    load_sb(sb + pf, split=(sb + pf == NSB - 1))
                if sb >= 4:
                    nc.sync.dma_start(out[sb - 4], stages.pop(sb - 4))
                xt = xt_tiles[sb]

                # q|k projections for both batches: one 2KB PSUM bank,
                # two accumulation groups of DoubleRow matmuls. scores(sb-1)
                # is emitted BETWEEN the groups so its exp/mask chain starts
                # ~1.5us earlier in the iteration (it is the longest serial
                # chain feeding next iteration's finale).
                qk_ps = qk_psp.tile([128, 2, T], F32, tag="qk")
                n = qk_terms * NCP

                def qk_group(bi):
                    i = 0
                    for xh_, wh_ in TERMS[:qk_terms]:
                        for cp in range(NCP):
                            nc.tensor.matmul(
                                qk_ps[:, bi],
                                lhsT=wqk_sb[:, 2 * cp : 2 * cp + 2],
                                rhs=xt[:, bi, xh_, 2 * cp : 2 * cp + 2, :],
                                start=(i == 0),
                                stop=(i == n - 1),
                                perf_mode=DR,
                            )
                            i += 1

                qk_group(0)
                qk_group(1)

                # q/k copies first in the DVE/ACT programs: they are the
                # critical arm feeding this superbatch's scores
                q_sb = sbp.tile([64, 2, T], BF16, tag="q_sb")
                k_sb = sbp.tile([64, 2, T], BF16, tag="k_sb")
                nc.vector.tensor_copy(q_sb, qk_ps[0:64])
                nc.scalar.copy(k_sb, qk_ps[64:128])


                # v projections: all four groups in one PSUM bank, single
                # fused ACT copy (emitted before exp in the ACT program)
                v_sb = vp.tile([128, 2, 2, D + 1], BF16, tag="v")
                v_ps = v_psp.tile([128, 2, 2, D], F32, tag="v_ps")
                n = v_terms * NCP
                for bi in range(2):
                    for st in range(2):
                        i = 0
                        for xh_, wh_ in TERMS[:v_terms]:
                            for cp in range(NCP):
                                nc.tensor.matmul(
                                    v_ps[:, bi, st],
                                    lhsT=xt[
                                        :, bi, xh_, 2 * cp : 2 * cp + 2,
                                        st * 128 : (st + 1) * 128,
                                    ],
                                    rhs=wv_sb[:, 2 * cp : 2 * cp + 2, wh_],
                                    start=(i == 0),
                                    stop=(i == n - 1),
                                    perf_mode=DR,
                                )
                                i += 1
                if vcopy_eng == 'dve' or sb >= NSB - 2:
                    nc.vector.tensor_copy(v_sb[:, :, :, 0:D], v_ps)
                elif vcopy_eng == 'pool':
                    nc.gpsimd.tensor_copy(v_sb[:, :, :, 0:D], v_ps)
                else:
                    nc.scalar.copy(v_sb[:, :, :, 0:D], v_ps)
                nc.gpsimd.memset(v_sb[:, :, :, D : D + 1], WS)

                # finale(sb-2) first: inputs are two iterations old
                if len(fin_q) >= 1 and pend_sc is not None:
                    final_stage(*fin_q.pop(0))
                if pend_sc is not None:
                    fin_q.append(scores_stage(*pend_sc))

                pend_sc = (sb, q_sb, k_sb, v_sb)

            # drain: scores(15), finals(14..15), stores for sb 12..15
            fin_q.append(scores_stage(*pend_sc, mask_mix=True))
            nc.sync.dma_start(out[NSB - 4], stages.pop(NSB - 4))
            final_stage(*fin_q.pop(0))
            nc.sync.dma_start(out[NSB - 3], stages.pop(NSB - 3))
            final_stage(*fin_q.pop(0))
            nl = stages.pop(NSB - 2)
            nc.sync.dma_start(out[NSB - 2][:, 0], nl[:, 0])
            nc.sync.dma_start(out[NSB - 2][:, 1], nl[:, 1])
            nc.sync.dma_start(out[NSB - 1], stages.pop(NSB - 1))
    nc.compile()
    return nc


def _pack_inputs(x, Wq, Wk, Wv):
    """Host-side layout/dtype prep: per-core [NSB,128,2,2,CCH,T] fp8 hi/lo x
    and the shared packed weight blob."""
    xt = np.ascontiguousarray(np.transpose(x, (0, 2, 1)))  # [B, C, T] f32
    xh = xt.astype(E4M3)
    xl = (xt - xh.astype(np.float32)).astype(E4M3)
    # [B, C, T] -> [B//2, 2, CCH, 128, T] -> stack hl -> [NSB*8, 128, 2, 2, CCH, T]
    def pack(a):
        return a.reshape(B // 2, 2, CCH, 128, T)
    ph, pl = pack(xh), pack(xl)
    xhl = np.stack([ph, pl], axis=2)  # [B//2, 2, 2, CCH, 128, T]
    xhl = np.ascontiguousarray(xhl.transpose(0, 4, 1, 2, 3, 5))

    def pack_w(W, m):
        w8 = W * WS
        wh = w8.astype(E4M3)
        wl = (w8 - wh.astype(np.float32)).astype(E4M3)
        return np.ascontiguousarray(
            np.stack(
                [wh.reshape(CCH, 128, m), wl.reshape(CCH, 128, m)], axis=2
            ).transpose(1, 0, 2, 3)
        )

    wqk = pack_w(np.concatenate([Wq, Wk], axis=1), 128)[:, :, 0]
    wv = pack_w(Wv, D)
    return np.ascontiguousarray(xhl), np.ascontiguousarray(wqk), wv


def kernel(x: np.ndarray, Wq: np.ndarray, Wk: np.ndarray, Wv: np.ndarray) -> np.ndarray:
    global LAST_RESULT
    x = np.asarray(x, dtype=np.float32)
    Wq = np.asarray(Wq, dtype=np.float32)
    Wk = np.asarray(Wk, dtype=np.float32)
    Wv = np.asarray(Wv, dtype=np.float32)

    xhl, wqk, wv = _pack_inputs(x, Wq, Wk, Wv)

    nc = _build()
    in_maps = [
        {"xhl": xhl[i * NSB : (i + 1) * NSB], "wqk_d": wqk, "wv_d": wv}
        for i in range(NCORES)
    ]
    res = run_bass_kernel_spmd(
        nc, in_maps, core_ids=list(range(NCORES)), trace=TRACE
    )
    LAST_RESULT = res
    # [NSB, 128, 2, 2, D] -> [NSB, 2, 2, 128, D] -> [BPC, T, D]
    outs = [
        np.ascontiguousarray(r["out"].transpose(0, 2, 3, 1, 4))
        .reshape(BPC, T, D)
        .astype(np.float32)
        for r in res.results
    ]
    return np.concatenate(outs, axis=0)


if __name__ == "__main__":
    x = np.random.randn(B, T, C).astype(np.float32)
    Wq = np.random.randn(C, D).astype(np.float32) * (C**-0.5)
    Wk = np.random.randn(C, D).astype(np.float32) * (C**-0.5)
    Wv = np.random.randn(C, D).astype(np.float32) * (C**-0.5)
    o = kernel(x, Wq, Wk, Wv)
    print(o.shape, o.dtype)



# revision 11
# speedup vs baseline: 1.3183x; 1.3183x over previous
"""Single-head causal attention (B=256, T=256, C=1024, D=64) on 8 TRN2 NeuronCores.

Data-parallel over batch (32 batches/core). v2 scheme halves the x DMA
traffic vs the fp8 hi/lo-pair baseline:

  * x ships as fp8-e4m3 xh for ALL positions plus the xl residual for only
    the first 64 sequence positions (5120B/partition/superbatch, one DMA).
    Early positions dominate both signal and error of causal attention
    (softmax over few values), so correcting v rows s<64 and q/k rows t<64
    recovers most of the accuracy of a full hi/lo pair at 56% of the bytes:
        q|k^T [128,T]: xh@(wh+wl)  + xl@wh on t<64      (rel err 1.06e-2
        v     [T,D]:   xh@(wvh+wvl) + xl@wvh on s<64     vs the 2e-2 gate)
    Weights are host-prescaled by 8 so their fp8 hi/lo splits stay normal;
    the 8x comes out in the exp scale and the fused denominator column.
  * Per-superbatch (2 batches): DMA 1820ns (load) + 182ns (store) vs PE
    ~1975ns -- DMA and PE are both ~98% busy (ridge).  DVE carries q/v
    copies + recip + finale mult; ACT carries k copy + exps; Pool masks.
  * Head: wqk ships as separate hi/lo DMAs and sb0 loads per batch
    (batch-major copy) so the first projections start ~1.3us in. Tail:
    sb15 also loads per batch and runs per-batch scores/finale/store
    chains with causal masks split across Pool and DVE.
"""

import numpy as np
import ml_dtypes

import concourse.bacc as bacc
import concourse.mybir as mybir
import concourse.tile as tile
from concourse.bass_utils import run_bass_kernel_spmd

B, T, C, D = 256, 256, 1024, 64
NCORES = 8
BPC = B // NCORES  # batches per core
NSB = BPC // 2  # superbatches (2 batches each)
CCH = C // 128  # contraction chunks
NCP = CCH // 2  # chunk pairs (DoubleRow contracts 2 chunks/instruction)
S0 = 64  # xl residual shipped for s < S0 only
WS = 8.0  # host weight prescale
SCALE = float(C) ** -0.5

BF16 = mybir.dt.bfloat16
F32 = mybir.dt.float32
F8 = mybir.dt.float8e4
E4M3 = ml_dtypes.float8_e4m3
DR = mybir.MatmulPerfMode.DoubleRow

TRACE = False
LAST_RESULT = None


def _build(pf=3):
    nc = bacc.Bacc(
        "TRN2", target_bir_lowering=False, debug=False, num_devices=NCORES
    )
    # x: [sb, partition, row, batch, quarter, 64] -- rows 0..7 hold xh chunk r
    # as [b, t(=4*64)], rows 8..9 hold xl (s<64) chunk 4*(r-8)+q as [b, q, s].
    xt_d = nc.dram_tensor("xt_d", [NSB, 128, 10, 2, 4, 64], F8, kind="ExternalInput")
    # sb0 and sb15 duplicated batch-major for per-batch head/tail loads
    xe_d = nc.dram_tensor("xe_d", [2, 2, 128, 10, 4, 64], F8, kind="ExternalInput")
    wqk_d = nc.dram_tensor("wqk_d", [2, 128, CCH, 128], F8, kind="ExternalInput")
    wv_d = nc.dram_tensor("wv_d", [128, CCH, 2, 64], F8, kind="ExternalInput")
    out = nc.dram_tensor("out", [NSB, 128, 2, 2, D], BF16, kind="ExternalOutput")

    with tile.TileContext(nc) as tc:
        with (
            tc.tile_pool(name="singles", bufs=1) as singles,
            tc.tile_pool(name="xp", bufs=pf + 1) as xp,
            tc.tile_pool(name="sbp", bufs=5) as sbp,
            tc.tile_pool(name="ep", bufs=9) as ep,
            tc.tile_pool(name="vp", bufs=6) as vp,
            tc.tile_pool(name="stp", bufs=5) as stp,
            tc.tile_pool(name="rp", bufs=3) as rp,
            tc.tile_pool(name="ocp", bufs=2) as ocp,
            tc.tile_pool(name="qk_ps", bufs=2, space="PSUM") as qk_psp,
            tc.tile_pool(name="sc_ps", bufs=3, space="PSUM") as sc_psp,
            tc.tile_pool(name="v_ps", bufs=2, space="PSUM") as v_psp,
            tc.tile_pool(name="o_ps", bufs=1, space="PSUM") as o_psp,
        ):
            # weight loads split so the first projection group (hi terms of
            # batch 0) can start as early as possible
            wqk_sb = singles.tile([128, 2, CCH, 128], F8)
            nc.sync.dma_start(wqk_sb[:, 0], wqk_d[0])
            # sb0 per-batch tiles (batch-major source)
            t0b = [singles.tile([128, 10, 4, 64], F8, name=f"t0b{b}") for b in range(2)]
            nc.sync.dma_start(t0b[0], xe_d[0, 0])
            nc.sync.dma_start(wqk_sb[:, 1], wqk_d[1])
            wv_sb = singles.tile([128, CCH, 2, 64], F8)
            nc.sync.dma_start(wv_sb, wv_d[:])
            nc.sync.dma_start(t0b[1], xe_d[0, 1])

            # causal triangle for DVE-side masking at the tail
            tri = singles.tile([128, 128], BF16)
            nc.gpsimd.memset(tri, 1.0)
            nc.gpsimd.affine_select(
                out=tri, in_=tri,
                compare_op=mybir.AluOpType.is_ge,
                fill=0.0, base=0, pattern=[[1, 128]], channel_multiplier=-1,
            )

            xt_tiles = {}
            t15b = [None, None]

            def load_sb(k):
                if k == NSB - 1:
                    t15b[0] = singles.tile([128, 10, 4, 64], F8, name="t15b0")
                    t15b[1] = singles.tile([128, 10, 4, 64], F8, name="t15b1")
                    nc.sync.dma_start(t15b[0], xe_d[1, 0])
                    nc.sync.dma_start(t15b[1], xe_d[1, 1])
                else:
                    t = xp.tile([128, 10, 2, 4, 64], F8, tag="xt")
                    nc.sync.dma_start(t, xt_d[k])
                    xt_tiles[k] = t

            # --- access helpers -------------------------------------------
            def xh_qk_rhs(sb, cp, b=None):
                """moving xh for the q|k projection; dim1 = chunk pair."""
                if sb in (0, NSB - 1):
                    tb = (t0b if sb == 0 else t15b)[b]
                    return tb[:, 2 * cp : 2 * cp + 2, :, :]
                return xt_tiles[sb][:, 2 * cp : 2 * cp + 2, :, :, :]

            def xl_ap(sb, cp, b):
                """xl chunk pair (2cp, 2cp+1) for batch b: [128, 2, 64]."""
                r, q = 8 + cp // 2, (2 * cp) % 4
                if sb in (0, NSB - 1):
                    tb = (t0b if sb == 0 else t15b)[b]
                    return tb[:, r, q : q + 2, :]
                return xt_tiles[sb][:, r, b, q : q + 2, :]

            def xh_v_lhsT(sb, cp, b, st):
                if sb in (0, NSB - 1):
                    tb = (t0b if sb == 0 else t15b)[b]
                    return tb[:, 2 * cp : 2 * cp + 2, 2 * st : 2 * st + 2, :]
                return xt_tiles[sb][:, 2 * cp : 2 * cp + 2, b, 2 * st : 2 * st + 2, :]

            # --- stages ---------------------------------------------------
            def emit_qk(sb):
                """q|k^T projections -> PSUM [128, 2, T] (q rows 0:64, k 64:128)."""
                qk_ps = qk_psp.tile([128, 2, T], F32, tag="qk")
                per_batch = sb in (0, NSB - 1)
                if per_batch:
                    for b in range(2):
                        i, n = 0, 2 * NCP + NCP
                        for h in range(2):
                            for cp in range(NCP):
                                nc.tensor.matmul(
                                    qk_ps[:, b],
                                    lhsT=wqk_sb[:, h, 2 * cp : 2 * cp + 2, :],
                                    rhs=xh_qk_rhs(sb, cp, b),
                                    start=(i == 0), stop=False, perf_mode=DR,
                                )
                                i += 1
                        for cp in range(NCP):
                            nc.tensor.matmul(
                                qk_ps[:, b, 0:S0],
                                lhsT=wqk_sb[:, 0, 2 * cp : 2 * cp + 2, :],
                                rhs=xl_ap(sb, cp, b),
                                start=False, stop=(i == n - 1), perf_mode=DR,
                            )
                            i += 1
                else:
                    i, n = 0, 2 * NCP + 2 * NCP
                    for h in range(2):
                        for cp in range(NCP):
                            nc.tensor.matmul(
                                qk_ps[:, :, :],
                                lhsT=wqk_sb[:, h, 2 * cp : 2 * cp + 2, :],
                                rhs=xh_qk_rhs(sb, cp),
                                start=(i == 0), stop=False, perf_mode=DR,
                            )
                            i += 1
                    for cp in range(NCP):
                        for b in range(2):
                            nc.tensor.matmul(
                                qk_ps[:, b, 0:S0],
                                lhsT=wqk_sb[:, 0, 2 * cp : 2 * cp + 2, :],
                                rhs=xl_ap(sb, cp, b),
                                start=False, stop=(i == n - 1), perf_mode=DR,
                            )
                            i += 1
                return qk_ps

            def emit_v(sb):
                """v -> bf16 [128, 2, 2, D+1] with the fused denominator col."""
                v_sb = vp.tile([128, 2, 2, D + 1], BF16, tag="v")
                v_ps = v_psp.tile([128, 2, 2, D], F32, tag="v_ps")
                for b in range(2):
                    for st in range(2):
                        i = 0
                        n = 2 * NCP + (NCP if st == 0 else 0)
                        for h in range(2):
                            for cp in range(NCP):
                                nc.tensor.matmul(
                                    v_ps[:, b, st],
                                    lhsT=xh_v_lhsT(sb, cp, b, st),
                                    rhs=wv_sb[:, 2 * cp : 2 * cp + 2, h, :],
                                    start=(i == 0), stop=(i == n - 1),
                                    perf_mode=DR,
                                )
                                i += 1
                        if st == 0:
                            for cp in range(NCP):
                                nc.tensor.matmul(
                                    v_ps[0:S0, b, 0, :],
                                    lhsT=xl_ap(sb, cp, b),
                                    rhs=wv_sb[:, 2 * cp : 2 * cp + 2, 0, :],
                                    start=False, stop=(i == n - 1),
                                    perf_mode=DR,
                                )
                                i += 1
                nc.scalar.copy(v_sb[:, :, :, 0:D], v_ps)
                nc.gpsimd.memset(v_sb[:, :, :, D : D + 1], WS)
                return v_sb

            def scores_stage(sb, q_sb, k_sb, bi, drain=False):
                """scores^T + exp + causal mask for one batch.
                Packed [128, 384]: cols 0:256 = (s<128, all t),
                256:384 = (s>=128, t>=128)."""
                sc_ps = sc_psp.tile([128, 3 * 128], F32, tag="sc")
                nc.tensor.matmul(
                    sc_ps[:, 0:T],
                    lhsT=k_sb[:, bi, 0:128],
                    rhs=q_sb[:, bi],
                    start=True, stop=True,
                )
                nc.tensor.matmul(
                    sc_ps[:, T : T + 128],
                    lhsT=k_sb[:, bi, 128:T],
                    rhs=q_sb[:, bi, 128:T],
                    start=True, stop=True,
                )
                expT = ep.tile([128, 3 * 128], BF16, tag="expT")
                nc.scalar.activation(
                    expT, sc_ps,
                    func=mybir.ActivationFunctionType.Exp,
                    scale=SCALE / (WS * WS),
                )
                for qi, quad in enumerate((0, 256)):
                    if drain and qi == 0:
                        # tail: halve the serial mask chain across engines
                        nc.vector.tensor_tensor(
                            expT[:, quad : quad + 128],
                            expT[:, quad : quad + 128],
                            tri, mybir.AluOpType.mult,
                        )
                    else:
                        nc.gpsimd.affine_select(
                            out=expT[:, quad : quad + 128],
                            in_=expT[:, quad : quad + 128],
                            compare_op=mybir.AluOpType.is_ge,
                            fill=0.0, base=0, pattern=[[1, 128]],
                            channel_multiplier=-1,
                        )
                return expT

            def final_mms(o2, bi, expT, v_sb):
                nc.tensor.matmul(
                    o2[:, bi, 0], lhsT=expT[:, 0:128], rhs=v_sb[:, bi, 0],
                    start=True, stop=True,
                )
                nc.tensor.matmul(
                    o2[:, bi, 1], lhsT=expT[:, 128:256], rhs=v_sb[:, bi, 0],
                    start=True, stop=False,
                )
                nc.tensor.matmul(
                    o2[:, bi, 1], lhsT=expT[:, 256:384], rhs=v_sb[:, bi, 1],
                    start=False, stop=True,
                )

            stages = {}

            def final_stage(sb, expTs, v_sb):
                """o' matmuls + softmax normalization, both batches fused.
                The normalize chain is spread ACT (PSUM->SBUF copy) ->
                DVE (recip, cheap SBUF read) -> Pool (multiply) to keep
                DVE under the DMA cadence."""
                o2 = o_psp.tile([128, 2, 2, D + 1], F32, tag="o_ps")
                for bi in range(2):
                    final_mms(o2, bi, expTs[bi], v_sb)
                o2c = ocp.tile([128, 2, 2, D + 1], F32, tag="o2c")
                nc.scalar.copy(o2c, o2)
                stages[sb] = stp.tile(
                    [128, 2, 2, D], BF16, tag="stage", name="stage"
                )
                recip = rp.tile([128, 2, 2], F32, tag="recip")
                nc.vector.reciprocal(recip, o2c[:, :, :, D])
                nc.gpsimd.tensor_tensor(
                    stages[sb],
                    o2c[:, :, :, 0:D],
                    recip[:, :, :, None].to_broadcast((128, 2, 2, D)),
                    mybir.AluOpType.mult,
                )

            # --- head loads ----------------------------------------------
            for k in range(1, min(pf, NSB)):
                load_sb(k)

            pend_sc = None  # (sb, q_sb, k_sb, v_sb) awaiting scores/exp/mask
            fin_q = []  # (sb, [expT_b0, expT_b1], v_sb) awaiting finale
            for sb in range(NSB):
                if sb + pf < NSB:
                    load_sb(sb + pf)
                if sb >= 4:
                    nc.sync.dma_start(out[sb - 4], stages.pop(sb - 4))

                qk_ps = emit_qk(sb)
                q_sb = sbp.tile([64, 2, T], BF16, tag="q_sb")
                k_sb = sbp.tile([64, 2, T], BF16, tag="k_sb")
                nc.vector.tensor_copy(q_sb, qk_ps[0:64])
                nc.vector.tensor_copy(k_sb, qk_ps[64:128])
                # scores(sb-1) right after the qk group in the PE program:
                # its exp->mask chain (ACT/Pool serial, ~1.9us) must complete
                # before finale(sb-1) runs in the NEXT iteration's PE stream.
                if pend_sc is not None:
                    psb, pq, pk, pv = pend_sc
                    fin_q.append(
                        (psb, [scores_stage(psb, pq, pk, bi) for bi in range(2)], pv)
                    )
                v_sb = emit_v(sb)
                # finale lag 3: expT(sb-3) masks are a full extra iteration
                # old, so the in-order PE stream never stalls on them
                if len(fin_q) >= 3:
                    final_stage(*fin_q.pop(0))
                pend_sc = (sb, q_sb, k_sb, v_sb)

            # --- drain: per-batch chains for the tail ---------------------
            psb, pq, pk, pv = pend_sc  # sb 15
            e15 = [scores_stage(psb, pq, pk, bi, drain=True) for bi in range(2)]
            nc.sync.dma_start(out[NSB - 4], stages.pop(NSB - 4))
            final_stage(*fin_q.pop(0))  # sb 13
            nc.sync.dma_start(out[NSB - 3], stages.pop(NSB - 3))
            final_stage(*fin_q.pop(0))  # sb 14
            nc.sync.dma_start(out[NSB - 2], stages.pop(NSB - 2))
            # sb15: per-batch finale -> recip -> mult -> store
            o2 = o_psp.tile([128, 2, 2, D + 1], F32, tag="o_ps")
            st15 = stp.tile([128, 2, 2, D], BF16, tag="stage")
            for bi in range(2):
                final_mms(o2, bi, e15[bi], pv)
                o2c = ocp.tile([128, 2, D + 1], F32, tag="o2c_b", name="o2c_b")
                nc.scalar.copy(o2c, o2[:, bi])
                recip = rp.tile([128, 2], F32, tag="recip_b")
                nc.vector.reciprocal(recip, o2c[:, :, D])
                nc.gpsimd.tensor_tensor(
                    st15[:, bi],
                    o2c[:, :, 0:D],
                    recip[:, :, None].to_broadcast((128, 2, D)),
                    mybir.AluOpType.mult,
                )
                nc.sync.dma_start(out[NSB - 1][:, bi], st15[:, bi])
    nc.compile()
    return nc


def _pack_inputs(x, Wq, Wk, Wv):
    """Host-side layout/dtype prep."""
    xt = np.ascontiguousarray(np.transpose(x, (0, 2, 1)))  # [B, C, T] f32
    xh = xt.astype(E4M3)
    xl = (xt[:, :, :S0] - xh[:, :, :S0].astype(np.float32)).astype(E4M3)

    # [B, 128, 10, 4, 64] batch-major packed array
    arr = np.empty((B, 128, 10, 4, 64), dtype=E4M3)
    arr[:, :, 0:8] = xh.reshape(B, 8, 128, 4, 64).transpose(0, 2, 1, 3, 4)
    arr[:, :, 8:10] = xl.reshape(B, 2, 4, 128, 64).transpose(0, 3, 1, 2, 4)

    def pack_w(W, m):
        w8 = W * WS
        wh = w8.astype(E4M3)
        wl = (w8 - wh.astype(np.float32)).astype(E4M3)
        return np.ascontiguousarray(
            np.stack(
                [wh.reshape(CCH, 128, m), wl.reshape(CCH, 128, m)], axis=2
            ).transpose(1, 0, 2, 3)
        )

    wqk = pack_w(np.concatenate([Wq, Wk], axis=1), 128)
    wqk = np.ascontiguousarray(wqk.transpose(2, 0, 1, 3))  # [2, 128, CCH, 128]
    wv = pack_w(Wv, D)  # [128, CCH, 2, 64]
    return arr, wqk, wv


def kernel(x: np.ndarray, Wq: np.ndarray, Wk: np.ndarray, Wv: np.ndarray) -> np.ndarray:
    global LAST_RESULT
    x = np.asarray(x, dtype=np.float32)
    Wq = np.asarray(Wq, dtype=np.float32)
    Wk = np.asarray(Wk, dtype=np.float32)
    Wv = np.asarray(Wv, dtype=np.float32)

    arr, wqk, wv = _pack_inputs(x, Wq, Wk, Wv)

    nc = _build()
    in_maps = []
    for i in range(NCORES):
        a = arr[i * BPC : (i + 1) * BPC]  # [32, 128, 10, 4, 64]
        # [NSB, 128, 10, 2, 4, 64]: batch inside row
        xt = np.ascontiguousarray(
            a.reshape(NSB, 2, 128, 10, 4, 64).transpose(0, 2, 3, 1, 4, 5)
        )
        xe = np.ascontiguousarray(
            np.stack([a[0:2], a[2 * NSB - 2 : 2 * NSB]], axis=0)
        )  # [2, 2, 128, 10, 4, 64]
        in_maps.append({"xt_d": xt, "xe_d": xe, "wqk_d": wqk, "wv_d": wv})
    res = run_bass_kernel_spmd(
        nc, in_maps, core_ids=list(range(NCORES)), trace=TRACE
    )
    LAST_RESULT = res
    # [NSB, 128, 2, 2, D] -> [NSB, 2, 2, 128, D] -> [BPC, T, D]
    outs = [
        np.ascontiguousarray(r["out"].transpose(0, 2, 3, 1, 4))
        .reshape(BPC, T, D)
        .astype(np.float32)
        for r in res.results
    ]
    return np.concatenate(outs, axis=0)


if __name__ == "__main__":
    x = np.random.randn(B, T, C).astype(np.float32)
    Wq = np.random.randn(C, D).astype(np.float32) * (C**-0.5)
    Wk = np.random.randn(C, D).astype(np.float32) * (C**-0.5)
    Wv = np.random.randn(C, D).astype(np.float32) * (C**-0.5)
    o = kernel(x, Wq, Wk, Wv)
    print(o.shape, o.dtype)


# revision 26
# speedup vs baseline: 1.3595x; 1.0313x over previous
"""Single-head causal attention (B=256, T=256, C=1024, D=64) on 8 TRN2 NeuronCores.

Data-parallel over batch (32 batches/core). v2 scheme halves the x DMA
traffic vs the fp8 hi/lo-pair baseline:

  * x ships as fp8-e4m3 xh for ALL positions plus the xl residual for only
    the first 64 sequence positions (5120B/partition/superbatch, one DMA).
    Early positions dominate both signal and error of causal attention
    (softmax over few values), so correcting v rows s<64 and q/k rows t<64
    recovers most of the accuracy of a full hi/lo pair at 56% of the bytes:
        q|k^T [128,T]: xh@(wh+wl)  + xl@wh on t<64      (rel err 1.06e-2
        v     [T,D]:   xh@(wvh+wvl) + xl@wvh on s<64     vs the 2e-2 gate)
    Weights are host-prescaled by 8 so their fp8 hi/lo splits stay normal;
    the 8x comes out in the exp scale and the fused denominator column.
  * Per-superbatch (2 batches): DMA 1820ns (load) + 182ns (store) vs PE
    ~1975ns -- DMA and PE are both ~98% busy (ridge).  DVE carries q/v
    copies + recip + finale mult; ACT carries k copy + exps; Pool masks.
  * Head: wqk ships as separate hi/lo DMAs and sb0 loads per batch
    (batch-major copy) so the first projections start ~1.3us in. Tail:
    sb15 also loads per batch and runs per-batch scores/finale/store
    chains with causal masks split across Pool and DVE.
"""

import numpy as np
import ml_dtypes

import concourse.bacc as bacc
import concourse.mybir as mybir
import concourse.tile as tile
from concourse.bass_utils import run_bass_kernel_spmd

B, T, C, D = 256, 256, 1024, 64
NCORES = 8
BPC = B // NCORES  # batches per core
NSB = BPC // 2  # superbatches (2 batches each)
CCH = C // 128  # contraction chunks
NCP = CCH // 2  # chunk pairs (DoubleRow contracts 2 chunks/instruction)
S0 = 64  # xl residual shipped for s < S0 only
WS = 8.0  # host weight prescale
SCALE = float(C) ** -0.5

BF16 = mybir.dt.bfloat16
F32 = mybir.dt.float32
F8 = mybir.dt.float8e4
E4M3 = ml_dtypes.float8_e4m3
DR = mybir.MatmulPerfMode.DoubleRow

TRACE = False
LAST_RESULT = None


def _build(pf=3):
    nc = bacc.Bacc(
        "TRN2", target_bir_lowering=False, debug=False, num_devices=NCORES
    )
    # x: [sb, partition, row, batch, quarter, 64] -- rows 0..7 hold xh chunk r
    # as [b, t(=4*64)], rows 8..9 hold xl (s<64) chunk 4*(r-8)+q as [b, q, s].
    xt_d = nc.dram_tensor("xt_d", [NSB, 128, 10, 2, 4, 64], F8, kind="ExternalInput")
    # sb0 and sb15 duplicated batch-major for per-batch head/tail loads
    xe_d = nc.dram_tensor("xe_d", [2, 2, 128, 10, 4, 64], F8, kind="ExternalInput")
    wqk_d = nc.dram_tensor("wqk_d", [2, 128, CCH, 128], F8, kind="ExternalInput")
    wv_d = nc.dram_tensor("wv_d", [128, CCH, 2, 64], F8, kind="ExternalInput")
    out = nc.dram_tensor("out", [NSB, 128, 2, 2, D], BF16, kind="ExternalOutput")

    with tile.TileContext(nc) as tc:
        with (
            tc.tile_pool(name="singles", bufs=1) as singles,
            tc.tile_pool(name="xp", bufs=pf + 1) as xp,
            tc.tile_pool(name="sbp", bufs=5) as sbp,
            tc.tile_pool(name="ep", bufs=9) as ep,
            tc.tile_pool(name="vp", bufs=6) as vp,
            tc.tile_pool(name="stp", bufs=5) as stp,
            tc.tile_pool(name="rp", bufs=3) as rp,
            tc.tile_pool(name="ocp", bufs=2) as ocp,
            tc.tile_pool(name="qk_ps", bufs=2, space="PSUM") as qk_psp,
            tc.tile_pool(name="sc_ps", bufs=3, space="PSUM") as sc_psp,
            tc.tile_pool(name="v_ps", bufs=2, space="PSUM") as v_psp,
            tc.tile_pool(name="o_ps", bufs=1, space="PSUM") as o_psp,
        ):
            # weight loads split so the first projection group (hi terms of
            # batch 0) can start as early as possible
            wqk_sb = singles.tile([128, 2, CCH, 128], F8)
            nc.sync.dma_start(wqk_sb[:, 0], wqk_d[0])
            # sb0 per-batch tiles (batch-major source)
            t0b = [singles.tile([128, 10, 4, 64], F8, name=f"t0b{b}") for b in range(2)]
            nc.sync.dma_start(t0b[0], xe_d[0, 0])
            nc.sync.dma_start(wqk_sb[:, 1], wqk_d[1])
            wv_sb = singles.tile([128, CCH, 2, 64], F8)
            nc.sync.dma_start(wv_sb, wv_d[:])
            nc.sync.dma_start(t0b[1], xe_d[0, 1])

            # causal triangle for DVE-side masking at the tail
            tri = singles.tile([128, 128], BF16)
            nc.gpsimd.memset(tri, 1.0)
            nc.gpsimd.affine_select(
                out=tri, in_=tri,
                compare_op=mybir.AluOpType.is_ge,
                fill=0.0, base=0, pattern=[[1, 128]], channel_multiplier=-1,
            )

            xt_tiles = {}
            t15b = [None, None]

            def load_sb(k):
                if k == NSB - 1:
                    t15b[0] = singles.tile([128, 10, 4, 64], F8, name="t15b0")
                    t15b[1] = singles.tile([128, 10, 4, 64], F8, name="t15b1")
                    nc.sync.dma_start(t15b[0], xe_d[1, 0])
                    nc.sync.dma_start(t15b[1], xe_d[1, 1])
                else:
                    t = xp.tile([128, 10, 2, 4, 64], F8, tag="xt")
                    nc.sync.dma_start(t, xt_d[k])
                    xt_tiles[k] = t

            # --- access helpers -------------------------------------------
            def xh_qk_rhs(sb, cp, b=None):
                """moving xh for the q|k projection; dim1 = chunk pair."""
                if sb in (0, NSB - 1):
                    tb = (t0b if sb == 0 else t15b)[b]
                    return tb[:, 2 * cp : 2 * cp + 2, :, :]
                return xt_tiles[sb][:, 2 * cp : 2 * cp + 2, :, :, :]

            def xl_ap(sb, cp, b):
                """xl chunk pair (2cp, 2cp+1) for batch b: [128, 2, 64]."""
                r, q = 8 + cp // 2, (2 * cp) % 4
                if sb in (0, NSB - 1):
                    tb = (t0b if sb == 0 else t15b)[b]
                    return tb[:, r, q : q + 2, :]
                return xt_tiles[sb][:, r, b, q : q + 2, :]

            def xh_v_lhsT(sb, cp, b, st):
                if sb in (0, NSB - 1):
                    tb = (t0b if sb == 0 else t15b)[b]
                    return tb[:, 2 * cp : 2 * cp + 2, 2 * st : 2 * st + 2, :]
                return xt_tiles[sb][:, 2 * cp : 2 * cp + 2, b, 2 * st : 2 * st + 2, :]

            # --- stages ---------------------------------------------------
            def emit_qk(sb):
                """q|k^T projections -> PSUM (q rows 0:64, k rows 64:128).
                sb 0 runs per-batch groups in one tile; sb 15 gets two
                per-batch TILES so batch 0's drain chain starts early."""
                if sb == NSB - 1:
                    tiles = []
                    for b in range(2):
                        qk_b = qk_psp.tile([128, T], F32, tag="qk", name="qk_b")
                        i, n = 0, 2 * NCP + NCP
                        for h in range(2):
                            for cp in range(NCP):
                                nc.tensor.matmul(
                                    qk_b,
                                    lhsT=wqk_sb[:, h, 2 * cp : 2 * cp + 2, :],
                                    rhs=xh_qk_rhs(sb, cp, b),
                                    start=(i == 0), stop=False, perf_mode=DR,
                                )
                                i += 1
                        for cp in range(NCP):
                            nc.tensor.matmul(
                                qk_b[:, 0:S0],
                                lhsT=wqk_sb[:, 0, 2 * cp : 2 * cp + 2, :],
                                rhs=xl_ap(sb, cp, b),
                                start=False, stop=(i == n - 1), perf_mode=DR,
                            )
                            i += 1
                        tiles.append(qk_b)
                    return tiles
                qk_ps = qk_psp.tile([128, 2, T], F32, tag="qk")
                if sb == 0:
                    for b in range(2):
                        i, n = 0, 2 * NCP + NCP
                        for h in range(2):
                            for cp in range(NCP):
                                nc.tensor.matmul(
                                    qk_ps[:, b],
                                    lhsT=wqk_sb[:, h, 2 * cp : 2 * cp + 2, :],
                                    rhs=xh_qk_rhs(sb, cp, b),
                                    start=(i == 0), stop=False, perf_mode=DR,
                                )
                                i += 1
                        for cp in range(NCP):
                            nc.tensor.matmul(
                                qk_ps[:, b, 0:S0],
                                lhsT=wqk_sb[:, 0, 2 * cp : 2 * cp + 2, :],
                                rhs=xl_ap(sb, cp, b),
                                start=False, stop=(i == n - 1), perf_mode=DR,
                            )
                            i += 1
                else:
                    i, n = 0, 2 * NCP + 2 * NCP
                    for h in range(2):
                        for cp in range(NCP):
                            nc.tensor.matmul(
                                qk_ps[:, :, :],
                                lhsT=wqk_sb[:, h, 2 * cp : 2 * cp + 2, :],
                                rhs=xh_qk_rhs(sb, cp),
                                start=(i == 0), stop=False, perf_mode=DR,
                            )
                            i += 1
                    for cp in range(NCP):
                        for b in range(2):
                            nc.tensor.matmul(
                                qk_ps[:, b, 0:S0],
                                lhsT=wqk_sb[:, 0, 2 * cp : 2 * cp + 2, :],
                                rhs=xl_ap(sb, cp, b),
                                start=False, stop=(i == n - 1), perf_mode=DR,
                            )
                            i += 1
                return qk_ps

            def emit_v(sb):
                """v -> bf16 [128, 2, 2, D+1] with the fused denominator col."""
                v_sb = vp.tile([128, 2, 2, D + 1], BF16, tag="v")
                v_ps = v_psp.tile([128, 2, 2, D], F32, tag="v_ps")
                for b in range(2):
                    for st in range(2):
                        i = 0
                        n = 2 * NCP + (NCP if st == 0 else 0)
                        for h in range(2):
                            for cp in range(NCP):
                                nc.tensor.matmul(
                                    v_ps[:, b, st],
                                    lhsT=xh_v_lhsT(sb, cp, b, st),
                                    rhs=wv_sb[:, 2 * cp : 2 * cp + 2, h, :],
                                    start=(i == 0), stop=(i == n - 1),
                                    perf_mode=DR,
                                )
                                i += 1
                        if st == 0:
                            for cp in range(NCP):
                                nc.tensor.matmul(
                                    v_ps[0:S0, b, 0, :],
                                    lhsT=xl_ap(sb, cp, b),
                                    rhs=wv_sb[:, 2 * cp : 2 * cp + 2, 0, :],
                                    start=False, stop=(i == n - 1),
                                    perf_mode=DR,
                                )
                                i += 1
                nc.scalar.copy(v_sb[:, :, :, 0:D], v_ps)
                nc.gpsimd.memset(v_sb[:, :, :, D : D + 1], WS)
                return v_sb

            def scores_stage(qa, ka):
                """scores^T + exp + causal mask for one batch (qa/ka are
                [64, 256] APs). Packed [128, 384]: cols 0:256 = (s<128,
                all t), 256:384 = (s>=128, t>=128)."""
                sc_ps = sc_psp.tile([128, 3 * 128], F32, tag="sc")
                nc.tensor.matmul(
                    sc_ps[:, 0:T], lhsT=ka[:, 0:128], rhs=qa,
                    start=True, stop=True,
                )
                nc.tensor.matmul(
                    sc_ps[:, T : T + 128], lhsT=ka[:, 128:T], rhs=qa[:, 128:T],
                    start=True, stop=True,
                )
                expT = ep.tile([128, 3 * 128], BF16, tag="expT")
                nc.scalar.activation(
                    expT, sc_ps,
                    func=mybir.ActivationFunctionType.Exp,
                    scale=SCALE / (WS * WS),
                )
                for quad in (0, 256):
                    nc.gpsimd.affine_select(
                        out=expT[:, quad : quad + 128],
                        in_=expT[:, quad : quad + 128],
                        compare_op=mybir.AluOpType.is_ge,
                        fill=0.0, base=0, pattern=[[1, 128]],
                        channel_multiplier=-1,
                    )
                return expT

            def final_mms(o2, bi, expT, v_sb):
                nc.tensor.matmul(
                    o2[:, bi, 0], lhsT=expT[:, 0:128], rhs=v_sb[:, bi, 0],
                    start=True, stop=True,
                )
                nc.tensor.matmul(
                    o2[:, bi, 1], lhsT=expT[:, 128:256], rhs=v_sb[:, bi, 0],
                    start=True, stop=False,
                )
                nc.tensor.matmul(
                    o2[:, bi, 1], lhsT=expT[:, 256:384], rhs=v_sb[:, bi, 1],
                    start=False, stop=True,
                )

            stages = {}

            def final_stage(sb, expTs, v_sb, pool=None, ptag="o_ps",
                            dve_norm=False, tail=False, stage=None):
                """o' matmuls + softmax normalization, both batches fused.
                Steady state spreads the normalize chain ACT (PSUM->SBUF
                copy) -> DVE (recip) -> Pool (multiply) to keep DVE under
                the DMA cadence. At drain (pool != None) ACT/Pool are the
                scarce engines, so recip+mult read PSUM directly on DVE.
                """
                drain = dve_norm or pool is not None
                o2 = (pool or o_psp).tile(
                    [128, 2, 2, D + 1], F32, tag=ptag, name="o2"
                )
                for bi in range(2):
                    final_mms(o2, bi, expTs[bi], v_sb)
                stages[sb] = stage if stage is not None else stp.tile(
                    [128, 2, 2, D], BF16, tag="stage", name="stage"
                )
                if drain:
                    recip = rp.tile([128, 2, 2], F32, tag="recip")
                    nc.vector.reciprocal(recip, o2[:, :, :, D])
                    nc.vector.tensor_tensor(
                        stages[sb],
                        o2[:, :, :, 0:D],
                        recip[:, :, :, None].to_broadcast((128, 2, 2, D)),
                        mybir.AluOpType.mult,
                    )
                    return
                o2c = ocp.tile([128, 2, 2, D + 1], F32, tag="o2c")
                nc.scalar.copy(o2c, o2)
                recip = rp.tile([128, 2, 2], F32, tag="recip")
                nc.vector.reciprocal(recip, o2c[:, :, :, D])
                # tail fins multiply on DVE (from SBUF): Pool is busy with
                # the sb14/15 mask chains there
                eng = nc.vector if tail else nc.gpsimd
                eng.tensor_tensor(
                    stages[sb],
                    o2c[:, :, :, 0:D],
                    recip[:, :, :, None].to_broadcast((128, 2, 2, D)),
                    mybir.AluOpType.mult,
                )

            # --- head loads ----------------------------------------------
            for k in range(1, min(pf, NSB)):
                load_sb(k)

            pend_sc = None  # (sb, q_sb, k_sb, v_sb) awaiting scores/exp/mask
            fin_q = []  # (sb, [expT_b0, expT_b1], v_sb) awaiting finale
            for sb in range(NSB):
                if sb + pf < NSB:
                    load_sb(sb + pf)
                if sb >= 4:
                    nc.sync.dma_start(out[sb - 4], stages.pop(sb - 4))

                # last iteration: scores(14) must precede the load-gated
                # qk(15) in the in-order PE stream, else its exp/mask chain
                # (and everything behind it) waits for the final load
                if sb == NSB - 1 and pend_sc is not None:
                    psb, pq, pk, pv = pend_sc
                    fin_q.append(
                        (psb, [scores_stage(pq[bi], pk[bi]) for bi in range(2)], pv)
                    )
                    pend_sc = None
                qk_ps = emit_qk(sb)
                if sb == NSB - 1:
                    q_aps, k_aps = [], []
                    for b in range(2):
                        q_b = sbp.tile([64, T], BF16, tag="q_b", name="q_b")
                        k_b = sbp.tile([64, T], BF16, tag="k_b", name="k_b")
                        nc.vector.tensor_copy(q_b, qk_ps[b][0:64])
                        nc.vector.tensor_copy(k_b, qk_ps[b][64:128])
                        q_aps.append(q_b[:])
                        k_aps.append(k_b[:])
                else:
                    q_sb = sbp.tile([64, 2, T], BF16, tag="q_sb")
                    k_sb = sbp.tile([64, 2, T], BF16, tag="k_sb")
                    nc.vector.tensor_copy(q_sb, qk_ps[0:64])
                    nc.vector.tensor_copy(k_sb, qk_ps[64:128])
                    q_aps = [q_sb[:, 0], q_sb[:, 1]]
                    k_aps = [k_sb[:, 0], k_sb[:, 1]]
                # scores(sb-1) right after the qk group in the PE program:
                # its exp->mask chain (ACT/Pool serial, ~1.9us) must complete
                # before finale(sb-1) runs a later iteration's PE stream.
                if pend_sc is not None:
                    psb, pq, pk, pv = pend_sc
                    fin_q.append(
                        (psb, [scores_stage(pq[bi], pk[bi]) for bi in range(2)], pv)
                    )
                    pend_sc = None
                v_sb = emit_v(sb)
                # finale lag 3: expT(sb-3) masks are a full extra iteration
                # old, so the in-order PE stream never stalls on them; the
                # last pop (fin 12) normalizes directly on DVE so its store
                # heads the drain queue without an ACT/Pool round-trip
                if len(fin_q) >= 3:
                    final_stage(*fin_q.pop(0), dve_norm=(sb == NSB - 1))
                pend_sc = (sb, q_aps, k_aps, v_sb)

            # --- drain ----------------------------------------------------
            # sb15 scores/exp/mask per batch first (the critical tail), then
            # the two pending finales on their own PSUM banks so their
            # normalize chains overlap, then sb15's per-batch finale+store.
            psb, pq, pk, v15 = pend_sc  # sb 15
            e15 = [scores_stage(pq[bi], pk[bi]) for bi in range(2)]
            nc.sync.dma_start(out[NSB - 4], stages.pop(NSB - 4))
            final_stage(*fin_q.pop(0), pool=o_psp)  # sb 13
            final_stage(*fin_q.pop(0), pool=v_psp, ptag="v_ps")  # sb 14
            # drain stores ride separate DGE queues so none waits behind an
            # earlier store whose data is still in flight
            nc.scalar.dma_start(out[NSB - 3], stages.pop(NSB - 3))
            nc.sync.dma_start(out[NSB - 2], stages.pop(NSB - 2))
            # sb15: per-batch finale -> recip -> mult -> store, normalize on
            # DVE (ACT/Pool are busy with the sb15 exp/mask chain)
            st15 = stp.tile([128, 2, 2, D], BF16, tag="stage")
            for bi in range(2):
                o2b = (sc_psp if bi == 0 else qk_psp).tile(
                    [128, 2, D + 1], F32, tag="sc" if bi == 0 else "qk",
                    name="o2b",
                )
                nc.tensor.matmul(
                    o2b[:, 0], lhsT=e15[bi][:, 0:128], rhs=v15[:, bi, 0],
                    start=True, stop=True,
                )
                nc.tensor.matmul(
                    o2b[:, 1], lhsT=e15[bi][:, 128:256], rhs=v15[:, bi, 0],
                    start=True, stop=False,
                )
                nc.tensor.matmul(
                    o2b[:, 1], lhsT=e15[bi][:, 256:384], rhs=v15[:, bi, 1],
                    start=False, stop=True,
                )
                recip = rp.tile([128, 2], F32, tag="recip_b")
                nc.vector.reciprocal(recip, o2b[:, :, D])
                nc.vector.tensor_tensor(
                    st15[:, bi],
                    o2b[:, :, 0:D],
                    recip[:, :, None].to_broadcast((128, 2, D)),
                    mybir.AluOpType.mult,
                )
                (nc.sync if bi == 0 else nc.scalar).dma_start(
                    out[NSB - 1][:, bi], st15[:, bi]
                )
    nc.compile()
    return nc


def _pack_inputs(x, Wq, Wk, Wv):
    """Host-side layout/dtype prep."""
    xt = np.ascontiguousarray(np.transpose(x, (0, 2, 1)))  # [B, C, T] f32
    xh = xt.astype(E4M3)
    xl = (xt[:, :, :S0] - xh[:, :, :S0].astype(np.float32)).astype(E4M3)

    # [B, 128, 10, 4, 64] batch-major packed array
    arr = np.empty((B, 128, 10, 4, 64), dtype=E4M3)
    arr[:, :, 0:8] = xh.reshape(B, 8, 128, 4, 64).transpose(0, 2, 1, 3, 4)
    arr[:, :, 8:10] = xl.reshape(B, 2, 4, 128, 64).transpose(0, 3, 1, 2, 4)

    def pack_w(W, m):
        w8 = W * WS
        wh = w8.astype(E4M3)
        wl = (w8 - wh.astype(np.float32)).astype(E4M3)
        return np.ascontiguousarray(
            np.stack(
                [wh.reshape(CCH, 128, m), wl.reshape(CCH, 128, m)], axis=2
            ).transpose(1, 0, 2, 3)
        )

    wqk = pack_w(np.concatenate([Wq, Wk], axis=1), 128)
    wqk = np.ascontiguousarray(wqk.transpose(2, 0, 1, 3))  # [2, 128, CCH, 128]
    wv = pack_w(Wv, D)  # [128, CCH, 2, 64]
    return arr, wqk, wv


def kernel(x: np.ndarray, Wq: np.ndarray, Wk: np.ndarray, Wv: np.ndarray) -> np.ndarray:
    global LAST_RESULT
    x = np.asarray(x, dtype=np.float32)
    Wq = np.asarray(Wq, dtype=np.float32)
    Wk = np.asarray(Wk, dtype=np.float32)
    Wv = np.asarray(Wv, dtype=np.float32)

    arr, wqk, wv = _pack_inputs(x, Wq, Wk, Wv)

    nc = _build()
    in_maps = []
    for i in range(NCORES):
        a = arr[i * BPC : (i + 1) * BPC]  # [32, 128, 10, 4, 64]
        # [NSB, 128, 10, 2, 4, 64]: batch inside row
        xt = np.ascontiguousarray(
            a.reshape(NSB, 2, 128, 10, 4, 64).transpose(0, 2, 3, 1, 4, 5)
        )
        xe = np.ascontiguousarray(
            np.stack([a[0:2], a[2 * NSB - 2 : 2 * NSB]], axis=0)
        )  # [2, 2, 128, 10, 4, 64]
        in_maps.append({"xt_d": xt, "xe_d": xe, "wqk_d": wqk, "wv_d": wv})
    res = run_bass_kernel_spmd(
        nc, in_maps, core_ids=list(range(NCORES)), trace=TRACE
    )
    LAST_RESULT = res
    # [NSB, 128, 2, 2, D] -> [NSB, 2, 2, 128, D] -> [BPC, T, D]
    outs = [
        np.ascontiguousarray(r["out"].transpose(0, 2, 3, 1, 4))
        .reshape(BPC, T, D)
        .astype(np.float32)
        for r in res.results
    ]
    return np.concatenate(outs, axis=0)


if __name__ == "__main__":
    x = np.random.randn(B, T, C).astype(np.float32)
    Wq = np.random.randn(C, D).astype(np.float32) * (C**-0.5)
    Wk = np.random.randn(C, D).astype(np.float32) * (C**-0.5)
    Wv = np.random.randn(C, D).astype(np.float32) * (C**-0.5)
    o = kernel(x, Wq, Wk, Wv)
    print(o.shape, o.dtype)
